# revision 21
# baseline (speedup 1.0000x reference)
"""GAE (generalized advantage estimation) Trainium2 kernel — PE matvec.

Problem: nn_CustomGAE — B=512, T=2048, D=64.
  value = obs @ W + b ; next_value = next_obs @ W + b
  td0 = reward + gamma*nd*next_value - value ; coef = gamma*lambda*nd
  A_t = td0_t + coef_t * A_{t+1}  (reverse scan over T, independent per traj)
  returns (advantage, value_target = advantage + value)

Sharding: pure data parallel over B across 8 cores (64 trajectories/core).
Each trajectory's T=2048 steps are split into H=2 halves -> 128 "rowtrajs"
(p = h*64 + b), tp=1024 timesteps each.

v3.2 design (PE matvec + u8 cast-DMA + merged input stream; ~44us/iter vs
151us for the v2 DVE-matvec kernel by interleaved repeat-ladder delta,
~5us above the IO-only floor measured the same way):
  * The matvec moves from DVE (v2: multiply + log-tree reduce, ~90us busy)
    to the previously idle TensorEngine. For each 512-timestep psum chunk,
    64 accumulating matmuls (one per trajectory b) with stationary
    stat_b [128, 128] that holds W in column b (partitions 0-63, h=0 d-planes)
    and column 64+b (partitions 64-127, h=1 d-planes) build the full
    [128 rowtraj, 512 t] value tile directly in PSUM:
        psum[b, t]    += sum_d W[d] * obs[(0,b), t, d]
        psum[64+b, t] += sum_d W[d] * obs[(1,b), t, d]
    (all other rows accumulate exact zeros; fp32 PSUM accumulation). PE
    cost ~55us, hidden under the DMA floor. DVE runs only the f32
    epilogue + scans (~11us busy).
  * obs/nobs quantized to uint8 on host (q = round(x/s) + 128, global
    scale), SWDGE cast-DMA (u8 DRAM -> bf16 SBUF) halves HBM traffic vs
    bf16; the binding resource becomes the SBUF-AXI write side. The scale
    is applied in the epilogue (vb = psum*sA + bA', bA' = b -
    128*sA*sum(W_bf16)), so the PE path is dtype-agnostic. End-to-end rel
    err ~1.1e-2 (gate 2e-2); fp8e4m3 instead would be 2.1e-2 - fails.
  * obs and nobs are interleaved per DMA group in ONE dram tensor
    [d-on-partition, (chunk, group, obs|nobs, traj, t) on free]: a single
    dma_start feeds a group's 2*GP matmuls with 8KB(u8)/partition
    contiguous descriptors. Halving the SWDGE DMA count this way cut both
    the kernel and its IO floor by ~6us (the single gpsimd queue's
    per-DMA descriptor-generation overhead is a real cost).
  * Chunks are processed latest-time-first and the backward scan chains
    chunk-by-chunk (initial = previous chunk's first column), so scans +
    output DMAs overlap the stream; only the lo-half (earlier-times) scan
    chain + the tiny cross-partition boundary DMA sit in the tail.
  * rw and the adv/tgt outputs are bf16 (host upcasts) to shave DMA bytes.
"""

import sys

sys.path.insert(0, "/opt/trn_rl_repo")

from contextlib import ExitStack

import ml_dtypes
import numpy as np

import concourse.bacc as bacc
import concourse.mybir as mybir
import concourse.tile as tile
from concourse.bass_utils import run_bass_kernel_spmd

GAMMA = 0.99
LMBDA = 0.95

B, T, D = 512, 2048, 64
NCORES = 8
BL = B // NCORES  # 64 trajectories per core
H = 2  # trajectory halves stacked on partitions -> 128 partitions
P = H * BL  # 128
F32 = mybir.dt.float32
BF16 = mybir.dt.bfloat16
U8 = mybir.dt.uint8
BF_NP = ml_dtypes.bfloat16

# Final kernel configuration (shared by build_program and shard_inputs).
# in_u8: ship obs/nobs as uint8 + SWDGE cast-DMA; gsz: trajectory pairs per
# DMA group (obs+nobs interleaved in one dram tensor -> one dma_start each).
CONFIG = dict(in_u8=True, gsz=8)

# Results of the last hardware run, for test harnesses.
LAST_RESULTS = None

mult = mybir.AluOpType.mult
add = mybir.AluOpType.add
sub = mybir.AluOpType.subtract


def _build_iter(
    nc, opool, pvpool, pnpool, ppool, dpool,
    stat, sA_t, bA_t, sB_t, bB_t, bnd,
    obsnb_d, rw_d, dn_d, adv_d, tgt_d,
    tp, CS, GP, in_eng, oeng, nocompute,
    out_bf16, rw_bf16, chain_scan,
):
    """One full pass: stream groups, matvec on PE, per-chunk epilogue, scan."""
    NCH = tp // CS
    ngrp = BL // GP
    GC = 2 * GP * CS  # group columns: [obs pairs | nobs pairs] interleaved
    ODT = BF16 if out_bf16 else F32
    RDT = BF16 if rw_bf16 else F32

    # persistent [P, tp] state
    vb = ppool.tile([P, tp], F32)     # value = obs@W + b
    coefc = ppool.tile([P, tp], F32)  # gamma*lambda*nd
    gq = ppool.tile([P, tp], F32)     # gamma*nd
    td0 = ppool.tile([P, tp], F32)
    adv = ppool.tile([P, tp], ODT)
    tgt = ppool.tile([P, tp], ODT)
    rw_t = ppool.tile([P, tp], RDT)
    dn_t = ppool.tile([P, tp], U8)

    if nocompute:
        # IO-only build to measure the DMA floor.
        for c in range(NCH):
            for g in range(ngrp):
                gt = opool.tile([P, GC], BF16)
                js = slice((c * ngrp + g) * GC, (c * ngrp + g + 1) * GC)
                in_eng.dma_start(gt[:], obsnb_d.ap()[:, js])
        nc.sync.dma_start(rw_t[:], rw_d.ap())
        nc.sync.dma_start(dn_t[:], dn_d.ap())
        oeng.dma_start(adv_d.ap(), rw_t[:])
        oeng.dma_start(tgt_d.ap(), rw_t[:])
        return

    hi = slice(BL, 2 * BL)
    lo = slice(0, BL)
    # chunks processed latest-time first so the hi-half scan can chain
    # chunk-by-chunk while earlier chunks still stream
    corder = list(range(NCH - 1, -1, -1)) if chain_scan else list(range(NCH))

    first = True
    for idx, c in enumerate(corder):
        psv = pvpool.tile([P, CS], F32)
        psn = pnpool.tile([P, CS], F32)
        cs_ = slice(c * CS, (c + 1) * CS)
        for g in range(ngrp):
            gt = opool.tile([P, GC], BF16)
            js = slice((c * ngrp + g) * GC, (c * ngrp + g + 1) * GC)
            in_eng.dma_start(gt[:], obsnb_d.ap()[:, js])
            if first:
                # on an independent queue; lands during the first group DMA
                nc.sync.dma_start(rw_t[:], rw_d.ap())
                nc.sync.dma_start(dn_t[:], dn_d.ap())
                ndf = dpool.tile([P, tp], F32)
                nc.vector.tensor_copy(ndf[:], dn_t[:])  # u8 -> f32
                nc.scalar.activation(
                    gq[:], ndf[:], mybir.ActivationFunctionType.Copy,
                    bias=GAMMA, scale=-GAMMA,
                )
                nc.scalar.activation(
                    coefc[:], ndf[:], mybir.ActivationFunctionType.Copy,
                    bias=GAMMA * LMBDA, scale=-GAMMA * LMBDA,
                )
                first = False
            for k2 in range(GP):
                k = g * GP + k2
                lhsT = stat[:, k * P : (k + 1) * P]
                nc.tensor.matmul(
                    psv[:], lhsT, gt[:, k2 * CS : (k2 + 1) * CS],
                    start=(k == 0), stop=(k == BL - 1),
                )
                nc.tensor.matmul(
                    psn[:], lhsT, gt[:, (GP + k2) * CS : (GP + k2 + 1) * CS],
                    start=(k == 0), stop=(k == BL - 1),
                )

        # ---- per-chunk epilogue: td0 = rw + g*(nv*sB+bB) - (v*sA+bA) ----
        nc.vector.tensor_scalar(
            vb[:, cs_], psv[:], sA_t[:, 0:1], bA_t[:, 0:1], op0=mult, op1=add
        )
        nvb = dpool.tile([P, CS], F32)
        nc.vector.tensor_scalar(
            nvb[:], psn[:], sB_t[:, 0:1], bB_t[:, 0:1], op0=mult, op1=add
        )
        q = dpool.tile([P, CS], F32)
        nc.vector.tensor_tensor(out=q[:], in0=gq[:, cs_], in1=nvb[:], op=mult)
        s_t = dpool.tile([P, CS], F32)
        nc.gpsimd.tensor_tensor(out=s_t[:], in0=rw_t[:, cs_], in1=vb[:, cs_], op=sub)
        nc.vector.tensor_tensor(out=td0[:, cs_], in0=q[:], in1=s_t[:], op=add)

        if chain_scan:
            # hi-half scan of this chunk (reverse over time), chained from
            # the previously scanned (later-time) chunk's first column
            init = 0.0 if idx == 0 else adv[hi, (c + 1) * CS : (c + 1) * CS + 1]
            nc.vector.tensor_tensor_scan(
                out=adv[hi, cs_][:, ::-1], data0=coefc[hi, cs_][:, ::-1],
                data1=td0[hi, cs_][:, ::-1], initial=init, op0=mult, op1=add,
            )
            nc.vector.tensor_tensor(
                out=tgt[hi, cs_], in0=adv[hi, cs_], in1=vb[hi, cs_], op=add
            )
            oeng.dma_start(adv_d.ap()[hi, cs_], adv[hi, cs_])
            oeng.dma_start(tgt_d.ap()[hi, cs_], tgt[hi, cs_])

    if chain_scan:
        # lo half: boundary A(tp) crosses partitions via a tiny DMA, then
        # chunk-chained scans as above
        oeng.dma_start(bnd[:], adv[hi, 0:1])
        for idx, c in enumerate(corder):
            cs_ = slice(c * CS, (c + 1) * CS)
            init = (
                bnd[:, 0:1] if idx == 0
                else adv[lo, (c + 1) * CS : (c + 1) * CS + 1]
            )
            nc.vector.tensor_tensor_scan(
                out=adv[lo, cs_][:, ::-1], data0=coefc[lo, cs_][:, ::-1],
                data1=td0[lo, cs_][:, ::-1], initial=init, op0=mult, op1=add,
            )
            nc.vector.tensor_tensor(
                out=tgt[lo, cs_], in0=adv[lo, cs_], in1=vb[lo, cs_], op=add
            )
            oeng.dma_start(adv_d.ap()[lo, cs_], adv[lo, cs_])
            oeng.dma_start(tgt_d.ap()[lo, cs_], tgt[lo, cs_])
        return

    # ---- unchained: backward scan, second half (later timesteps) first ----
    nc.vector.tensor_tensor_scan(
        out=adv[hi, ::-1], data0=coefc[hi, ::-1], data1=td0[hi, ::-1],
        initial=0.0, op0=mult, op1=add,
    )
    nc.vector.tensor_tensor(out=tgt[hi, :], in0=adv[hi, :], in1=vb[hi, :], op=add)
    oeng.dma_start(bnd[:], adv[hi, 0:1])
    oeng.dma_start(adv_d.ap()[hi, :], adv[hi, :])
    oeng.dma_start(tgt_d.ap()[hi, :], tgt[hi, :])
    nc.vector.tensor_tensor_scan(
        out=adv[lo, ::-1], data0=coefc[lo, ::-1], data1=td0[lo, ::-1],
        initial=bnd[:, 0:1], op0=mult, op1=add,
    )
    nc.vector.tensor_tensor(out=tgt[lo, :], in0=adv[lo, :], in1=vb[lo, :], op=add)
    oeng.dma_start(adv_d.ap()[lo, :], adv[lo, :])
    oeng.dma_start(tgt_d.ap()[lo, :], tgt[lo, :])


def build_program(
    t_total=T, repeat=1, nocompute=False, bench_internal=False,
    in_u8=None, gsz=None, obufs=4, psbufs=2,
    out_scalar=True, out_bf16=True, rw_bf16=True, chain_scan=True,
):
    """Build the per-core Bass program (all 8 cores run it SPMD on their own
    shard). Input DRAM layout: obs and nobs interleaved per DMA group,
    [p = (h,d), j = (chunk, group, obs|nobs, traj, t)], so one dma_start
    feeds a group's 2*GP matmuls and the moving operand for (pair k, chunk
    c) is a contiguous column slice. repeat>1 re-runs the pipeline inside
    one NEFF for delta-timing; bench_internal makes the big input Internal
    DRAM so bench invocations are cheap."""
    if in_u8 is None:
        in_u8 = CONFIG["in_u8"]
    if gsz is None:
        gsz = CONFIG["gsz"]
    tp = t_total // H  # timesteps per rowtraj
    CS = min(512, tp)  # psum chunk columns
    assert tp % CS == 0
    GP = gsz  # pairs per DMA group
    assert BL % GP == 0

    nc = bacc.Bacc(
        "TRN2", target_bir_lowering=False, debug=False, enable_asserts=False
    )

    big_kind = "Internal" if bench_internal else "ExternalInput"
    obsnb_d = nc.dram_tensor(
        "obsnb", [P, 2 * tp * D], U8 if in_u8 else BF16, kind=big_kind
    )
    if nocompute:
        assert out_bf16 == rw_bf16, "nocompute writes rw_t to the outputs"
    ODT = BF16 if out_bf16 else F32
    RDT = BF16 if rw_bf16 else F32
    rw_d = nc.dram_tensor("rw", [P, tp], RDT, kind="ExternalInput")
    dn_d = nc.dram_tensor("dn", [P, tp], U8, kind="ExternalInput")
    wbf_d = nc.dram_tensor("wbf", [D], BF16, kind="ExternalInput")
    sA_d = nc.dram_tensor("sA", [1], F32, kind="ExternalInput")
    bA_d = nc.dram_tensor("bA", [1], F32, kind="ExternalInput")
    sB_d = nc.dram_tensor("sB", [1], F32, kind="ExternalInput")
    bB_d = nc.dram_tensor("bB", [1], F32, kind="ExternalInput")
    adv_d = nc.dram_tensor("adv", [P, tp], ODT, kind="ExternalOutput")
    tgt_d = nc.dram_tensor("tgt", [P, tp], ODT, kind="ExternalOutput")

    # cast DMAs (u8 -> bf16) must go through SWDGE (gpsimd)
    in_eng = nc.gpsimd if in_u8 else nc.sync

    with tile.TileContext(nc) as tc_ctx, ExitStack() as ctx:
        cpool = ctx.enter_context(tc_ctx.tile_pool(name="const", bufs=1))
        opool = ctx.enter_context(tc_ctx.tile_pool(name="og", bufs=obufs))
        pvpool = ctx.enter_context(
            tc_ctx.tile_pool(name="psv", bufs=psbufs, space="PSUM")
        )
        pnpool = ctx.enter_context(
            tc_ctx.tile_pool(name="psn", bufs=psbufs, space="PSUM")
        )
        ppool = ctx.enter_context(tc_ctx.tile_pool(name="pers", bufs=1))
        dpool = ctx.enter_context(tc_ctx.tile_pool(name="dbl", bufs=2))

        # W replicated to both partition halves: w128[p] = W[p % 64]
        w128 = cpool.tile([P, 1], BF16)
        nc.sync.dma_start(w128[:], wbf_d.ap().unsqueeze(0).broadcast_to([H, D]))
        sA_t = cpool.tile([P, 1], F32)
        nc.sync.dma_start(sA_t[:], sA_d.ap().unsqueeze(0).broadcast_to([P, 1]))
        bA_t = cpool.tile([P, 1], F32)
        nc.sync.dma_start(bA_t[:], bA_d.ap().unsqueeze(0).broadcast_to([P, 1]))
        sB_t = cpool.tile([P, 1], F32)
        nc.sync.dma_start(sB_t[:], sB_d.ap().unsqueeze(0).broadcast_to([P, 1]))
        bB_t = cpool.tile([P, 1], F32)
        nc.sync.dma_start(bB_t[:], bB_d.ap().unsqueeze(0).broadcast_to([P, 1]))

        bnd = cpool.tile([BL, 1], ODT)

        # Stationary block: stat_k = stat[:, 128k:128k+128] has W at
        # column k (partitions 0-63) and column 64+k (partitions 64-127),
        # zeros elsewhere -> matmul k accumulates rowtraj (0,k) into psum
        # row k and rowtraj (1,k) into row 64+k, adding zero to the rest.
        stat = cpool.tile([P, BL * P], BF16)
        nc.vector.memset(stat[:], 0.0)
        nc.vector.tensor_copy(
            stat[0:BL, 0 : (BL - 1) * (P + 1) + 1 : P + 1],
            w128[0:BL, 0:1].broadcast_to([BL, BL]),
        )
        nc.vector.tensor_copy(
            stat[BL:P, BL : BL + (BL - 1) * (P + 1) + 1 : P + 1],
            w128[BL:P, 0:1].broadcast_to([BL, BL]),
        )

        oeng = nc.scalar if out_scalar else nc.sync

        for _rep in range(repeat):
            _build_iter(
                nc, opool, pvpool, pnpool, ppool, dpool,
                stat, sA_t, bA_t, sB_t, bB_t, bnd,
                obsnb_d, rw_d, dn_d, adv_d, tgt_d,
                tp, CS, GP, in_eng, oeng, nocompute,
                out_bf16, rw_bf16, chain_scan,
            )

    nc.finalize()
    return nc


_NC_CACHE = None


def _get_nc():
    global _NC_CACHE
    if _NC_CACHE is None:
        _NC_CACHE = build_program()
    return _NC_CACHE


def _swizzle_big(x, t_total=T):
    """[BL, t_total, D] -> [P, (t_total//H)*D] with p=(h,d), j=(c,k,t)."""
    tp = t_total // H
    CS = min(512, tp)
    NCH = tp // CS
    return np.ascontiguousarray(
        x.reshape(BL, H, NCH, CS, D)
        .transpose(1, 4, 2, 0, 3)
        .reshape(P, tp * D)
    )


def _merge_big(o_sw, n_sw, t_total=T, gsz=None):
    """Interleave swizzled obs/nobs per DMA group:
    [P, (c,k,t)] x2 -> [P, (c, g, {obs,nobs}, k2, t)]."""
    if gsz is None:
        gsz = CONFIG["gsz"]
    tp = t_total // H
    CS = min(512, tp)
    NCH = tp // CS
    ngrp = BL // gsz
    o4 = o_sw.reshape(P, NCH, ngrp, gsz * CS)
    n4 = n_sw.reshape(P, NCH, ngrp, gsz * CS)
    return np.ascontiguousarray(
        np.stack([o4, n4], axis=3).reshape(P, 2 * tp * D)
    )


def _hmajor(x, tp_cols):
    """[BL, H*tp_cols] row-major -> [H*BL, tp_cols] with row p = h*BL + b."""
    return np.ascontiguousarray(
        x.reshape(BL, H, tp_cols).transpose(1, 0, 2).reshape(H * BL, tp_cols)
    )


def _unhmajor(y):
    """Inverse of _hmajor for outputs: [H*BL, tp] -> [BL, H*tp]."""
    tp = y.shape[1]
    return y.reshape(H, BL, tp).transpose(1, 0, 2).reshape(BL, H * tp)


def _quant_u8(x):
    """Symmetric uint8 quantization: q = round(x/s) + 128, s from global max."""
    s = float(np.abs(x).max()) / 127.0
    q = np.clip(np.rint(x / np.float32(s)), -127, 127) + 128.0
    return q.astype(np.uint8), np.float32(s)


def shard_inputs(obs, next_obs, reward, done, W, b):
    """Split full inputs into the 8 per-core input maps."""
    in_u8 = CONFIG["in_u8"]
    obs = np.asarray(obs, dtype=np.float32).reshape(B, T, D)
    nobs = np.asarray(next_obs, dtype=np.float32).reshape(B, T, D)
    rw = np.asarray(reward, dtype=np.float32).reshape(B, T)
    dn = np.asarray(done).astype(np.uint8, copy=False).reshape(B, T)
    w_np = np.ascontiguousarray(np.asarray(W, dtype=np.float32)).reshape(D)
    b_np = float(np.ascontiguousarray(np.asarray(b, dtype=np.float32)).reshape(1)[0])

    w_bf = w_np.astype(BF_NP)
    sum_wbf = float(w_bf.astype(np.float32).sum())

    if in_u8:
        obs_q, sA = _quant_u8(obs)
        bA = np.float32(b_np - 128.0 * sA * sum_wbf)
        nobs_q, sB = _quant_u8(nobs)
        bB = np.float32(b_np - 128.0 * sB * sum_wbf)
    else:
        obs_q, sA, bA = obs.astype(BF_NP), np.float32(1.0), np.float32(b_np)
        nobs_q, sB, bB = nobs.astype(BF_NP), np.float32(1.0), np.float32(b_np)

    tp = T // H
    in_maps = []
    for i in range(NCORES):
        sl = slice(i * BL, (i + 1) * BL)
        in_maps.append(
            {
                "obsnb": _merge_big(
                    _swizzle_big(obs_q[sl]), _swizzle_big(nobs_q[sl])
                ),
                "rw": _hmajor(rw[sl], tp).astype(BF_NP),
                "dn": _hmajor(dn[sl], tp),
                "wbf": w_bf,
                "sA": np.asarray([sA], np.float32),
                "bA": np.asarray([bA], np.float32),
                "sB": np.asarray([sB], np.float32),
                "bB": np.asarray([bB], np.float32),
            }
        )
    return in_maps


def gather_outputs(results):
    advantage = np.concatenate(
        [_unhmajor(np.asarray(r["adv"], np.float32)) for r in results], axis=0
    ).reshape(B, T, 1)
    value_target = np.concatenate(
        [_unhmajor(np.asarray(r["tgt"], np.float32)) for r in results], axis=0
    ).reshape(B, T, 1)
    return advantage, value_target


def kernel(obs, next_obs, reward, done, W, b):
    global LAST_RESULTS
    nc = _get_nc()
    in_maps = shard_inputs(obs, next_obs, reward, done, W, b)
    res = run_bass_kernel_spmd(nc, in_maps, core_ids=list(range(NCORES)))
    LAST_RESULTS = res
    return gather_outputs(res.results)
